# revision 5
# baseline (speedup 1.0000x reference)
"""Trainium2 Bass kernel for a 1-layer LSTM (B=2048, T=512, I=4, H=64) + FC (O=4).

Sharding: data-parallel over batch across 8 NeuronCores (256 examples/core);
the tiny LSTM/FC weights are replicated.

On-core layout: SBUF partitions carry hidden/gate rows, the free dimension
carries batch.  The 256 local examples form two groups of 128, stacked in the
partition dimension (group 0 -> rows 0-63, group 1 -> rows 64-127) so every
ScalarE/VectorE instruction runs with all 128 lanes busy.

v2 step structure (critical cycle trimmed vs the v1 baseline):
  - All four gates of a step accumulate into ONE PSUM bank, laid out
    [f | i | g | o] (128 fp32 columns each); banks alternate per step so the
    next step's x-part matmuls never wait on this step's activation reads.
  - tanh(g) is folded into the sigmoid pass: the g-gate pre-activation is
    built DOUBLED (weights x2), so sigmoid gives g' = sigmoid(2g) and
    tanh(g) = 2g' - 1.  One ACT instruction covers sigmoid(i|2g) (contiguous
    PSUM read), and sigmoid(f) runs as its own earlier instruction so only
    the f-gate h-matmul gates the first activation.
  - Cell state is carried halved (Ct = c/2):  Ct' = sf*Ct + (g'-0.5)*si,
    where the second product is ONE scalar_tensor_tensor op.
  - tanh(c) = 2*sigmoid(4*Ct) - 1: the ACT's free input scale computes
    s = sigmoid(4*Ct), and the recurrent state fed to the PE is
    m = (s - 0.5) * so = h/2 (one stt op); the x2 is folded into W_hh / W_fc.
  per-step cycle: mm_f -> sig(f) -> sig(i|2g) -> stt v -> add -> sig(4Ct)
                  -> stt m, with sig(o), w = sf*Ct, mm_i/g/o off-cycle.

Measured baseline (v1): 2436 ns/step, 1.247 ms total.
"""

from contextlib import ExitStack

import numpy as np

import concourse.bass as bass
import concourse.tile as tile
from concourse import bacc, mybir
from concourse.bass_utils import run_bass_kernel_spmd

F32 = mybir.dt.float32
BF16 = mybir.dt.bfloat16
AF = mybir.ActivationFunctionType
ALU = mybir.AluOpType

H, I, O = 64, 4, 4
B, T_FULL = 2048, 512
NCORES = 8
BLOC = B // NCORES          # 256 examples per core
NG = 128                    # batch per group (2 groups per core)
KX = 2 * (1 + I)            # 10 rows of ox2: [1; x_g0; 1; x_g1]

USE_BF16 = True

# Gate order within the PSUM bank: [f | i | g | o] at 128-col offsets.
# PyTorch row order in W_ih/W_hh is (i, f, g, o).
GATE_ROWS = {"f": 1, "i": 0, "g": 2, "o": 3}   # which H-row-block of W_*
GATE_COL = {"f": 0, "i": 1, "g": 2, "o": 3}    # bank column block
# Weight scaling: x2 on all W_hh/W_fc (state is h/2), extra x2 on the g gate
# pre-activation (sigmoid(2g) trick).
S_HH = {"f": 2.0, "i": 2.0, "g": 4.0, "o": 2.0}
S_XB = {"f": 1.0, "i": 1.0, "g": 2.0, "o": 1.0}

BK = 512  # fp32 elements per PSUM bank (per partition)


def build_nc(T=T_FULL, use_bf16=None):
    if use_bf16 is None:
        use_bf16 = USE_BF16
    DT = BF16 if use_bf16 else F32
    nc = bacc.Bacc(
        "TRN2",
        target_bir_lowering=False,
        debug=False,
        enable_asserts=False,
        num_devices=NCORES,
    )

    xT2 = nc.dram_tensor("xT2", [T, KX, NG], DT, kind="ExternalInput")
    wh2 = nc.dram_tensor("wh2", [2 * H, 4, 2 * H], DT, kind="ExternalInput")
    wx2 = nc.dram_tensor("wx2", [KX, 4, 2 * H], DT, kind="ExternalInput")
    wfc2 = nc.dram_tensor("wfc2", [2 * H, 2 * O], DT, kind="ExternalInput")
    out = nc.dram_tensor("out", [2 * O, NG], F32, kind="ExternalOutput")

    with tile.TileContext(nc) as tc, ExitStack() as ctx:
        persist = ctx.enter_context(tc.tile_pool(name="persist", bufs=1))
        acts = ctx.enter_context(tc.tile_pool(name="acts", bufs=2))
        temps = ctx.enter_context(tc.tile_pool(name="temps", bufs=2))
        psum = ctx.enter_context(tc.tile_pool(name="psum", bufs=1, space="PSUM"))
        psum1 = ctx.enter_context(tc.tile_pool(name="psum1", bufs=1, space="PSUM"))

        # Weight loads: big recurrent weights on the sync queue; the small
        # x/FC weights on the otherwise-idle ScalarE HWDGE queue.
        wh2_sb = persist.tile([2 * H, 4, 2 * H], DT, tag="wh2")
        nc.sync.dma_start(wh2_sb[:], wh2[:])
        wx2_sb = persist.tile([KX, 4, 2 * H], DT, tag="wx2")
        nc.scalar.dma_start(wx2_sb[:], wx2[:])
        wfc2_sb = persist.tile([2 * H, 2 * O], DT, tag="wfc2")
        nc.scalar.dma_start(wfc2_sb[:], wfc2[:])

        # Persistent state: halved cell state Ct, double-buffered m = h/2.
        c_st = persist.tile([2 * H, NG], DT, tag="c")
        nc.vector.memset(c_st[:], 0.0)
        mbuf = []
        for j in range(2):
            mb = persist.tile([2 * H, NG], DT, tag=f"m{j}")
            nc.vector.memset(mb[:], 0.0)
            mbuf.append(mb)
        oxb = []
        for j in range(2):
            ox_t = persist.tile([KX, NG], DT, tag=f"ox{j}")
            oxb.append(ox_t)

        # One gate bank per step, two banks alternating. Regions (fp32 cols):
        # f@0:128, i@128:256, g@256:384, o@384:512.
        psG = psum.tile([2 * H, 2 * BK], F32, tag="psG")

        def bank(t):
            base = (t % 2) * BK
            return psG[:, base : base + BK]

        def region(t, gate):
            base = (t % 2) * BK + GATE_COL[gate] * NG
            return psG[:, base : base + NG]

        def x_parts(t):
            # x/bias contributions for step t into bank(t); first write
            # (start=True) clears the whole bank's has_written bits, the
            # rest land as fresh writes in their own column blocks.
            for k, gate in enumerate(("f", "i", "g", "o")):
                nc.tensor.matmul(
                    region(t, gate),
                    wx2_sb[:, GATE_COL[gate], :],
                    oxb[t % 2][:],
                    start=(k == 0),
                    stop=False,
                    skip_group_check=True,
                )

        # Prologue: x DMA + x-part matmuls for step 0.
        nc.sync.dma_start(oxb[0][:], xT2[0])
        x_parts(0)

        for t in range(T):
            mc = mbuf[t % 2]
            mn = mbuf[(t + 1) % 2]

            # Prefetch x for step t+1 and issue its x-part matmuls (they
            # run on the PE right after this step's h-wave).
            if t + 1 < T:
                nc.sync.dma_start(oxb[(t + 1) % 2][:], xT2[t + 1])

            # Recurrent matmuls: f first (it alone gates sig_f), o last
            # (sig_o is needed latest).
            for gate in ("f", "i", "g", "o"):
                nc.tensor.matmul(
                    region(t, gate),
                    wh2_sb[:, GATE_COL[gate], :],
                    mc[:],
                    start=False,
                    stop=True,
                    skip_group_check=True,
                )
            if t + 1 < T:
                x_parts(t + 1)

            # ACT: sigmoid(f) alone (only mm_f gates it), then the
            # contiguous sigmoid over [i|2g], then sigmoid(o).
            sf = acts.tile([2 * H, NG], DT, tag="sf")
            nc.scalar.activation(sf[:], region(t, "f"), AF.Sigmoid)
            sig = acts.tile([2 * H, 2 * NG], DT, tag="sig")  # [sig(i) | sig(2g)]
            base = (t % 2) * BK + NG
            nc.scalar.activation(sig[:], psG[:, base : base + 2 * NG], AF.Sigmoid)
            so = acts.tile([2 * H, NG], DT, tag="so")
            nc.scalar.activation(so[:], region(t, "o"), AF.Sigmoid)

            # DVE: w = sf*Ct (off-cycle, overlaps sig_ig), then
            # v = (g' - 0.5) * si, Ct = w + v.
            w = temps.tile([2 * H, NG], DT, tag="w")
            nc.vector.tensor_mul(w[:], sf[:], c_st[:])
            v = temps.tile([2 * H, NG], DT, tag="v")
            nc.vector.scalar_tensor_tensor(
                v[:], sig[:, NG : 2 * NG], 0.5, sig[:, 0:NG],
                op0=ALU.subtract, op1=ALU.mult,
            )
            nc.vector.tensor_add(c_st[:], w[:], v[:])

            # s = sigmoid(4*Ct) -> m' = (s - 0.5) * so  (= h/2)
            s4 = acts.tile([2 * H, NG], DT, tag="s4")
            nc.scalar.activation(s4[:], c_st[:], AF.Sigmoid, scale=4.0)
            nc.vector.scalar_tensor_tensor(
                mn[:], s4[:], 0.5, so[:],
                op0=ALU.subtract, op1=ALU.mult,
            )

        # Final FC: one matmul on m = h/2 (x2 folded into wfc2).
        hf = mbuf[T % 2]
        fc_ps = psum1.tile([2 * O, NG], F32, tag="fc")
        nc.tensor.matmul(fc_ps[:], wfc2_sb[:], hf[:], start=True, stop=True)
        fc_sb = temps.tile([2 * O, NG], F32, tag="fcsb")
        nc.vector.tensor_copy(fc_sb[:], fc_ps[:])
        nc.sync.dma_start(out[:], fc_sb[:])

    nc.compile()
    return nc


def prep_weights(W_ih, W_hh, b_ih, b_hh, W_fc, b_fc):
    bsum = (b_ih + b_hh).astype(np.float32)
    wh2 = np.zeros((2 * H, 4, 2 * H), np.float32)
    wx2 = np.zeros((KX, 4, 2 * H), np.float32)
    for gate, ch in GATE_COL.items():
        r = slice(GATE_ROWS[gate] * H, (GATE_ROWS[gate] + 1) * H)
        whT = S_HH[gate] * W_hh[r].T
        wh2[0:H, ch, 0:H] = whT
        wh2[H:, ch, H:] = whT
        wx2[0, ch, 0:H] = S_XB[gate] * bsum[r]
        wx2[1 : 1 + I, ch, 0:H] = S_XB[gate] * W_ih[r].T
        wx2[1 + I, ch, H:] = S_XB[gate] * bsum[r]
        wx2[2 + I :, ch, H:] = S_XB[gate] * W_ih[r].T
    wfc2 = np.zeros((2 * H, 2 * O), np.float32)
    wfc2[0:H, 0:O] = 2.0 * W_fc.T
    wfc2[H:, O:] = 2.0 * W_fc.T
    return wh2, wx2, wfc2


def make_in_maps(x, W_ih, W_hh, b_ih, b_hh, W_fc, b_fc, T=T_FULL, use_bf16=None):
    import ml_dtypes

    if use_bf16 is None:
        use_bf16 = USE_BF16
    npdt = ml_dtypes.bfloat16 if use_bf16 else np.float32
    wh2, wx2, wfc2 = prep_weights(W_ih, W_hh, b_ih, b_hh, W_fc, b_fc)
    wh2, wx2, wfc2 = (a.astype(npdt) for a in (wh2, wx2, wfc2))
    in_maps = []
    for core in range(NCORES):
        xc = x[core * BLOC : (core + 1) * BLOC, :T, :]  # [BLOC, T, I]
        xT = np.ascontiguousarray(xc.transpose(1, 2, 0))  # [T, I, BLOC]
        xT2 = np.empty((T, KX, NG), np.float32)
        xT2[:, 0, :] = 1.0
        xT2[:, 1 : 1 + I, :] = xT[:, :, 0:NG]
        xT2[:, 1 + I, :] = 1.0
        xT2[:, 2 + I :, :] = xT[:, :, NG : 2 * NG]
        in_maps.append(
            {"xT2": xT2.astype(npdt), "wh2": wh2, "wx2": wx2, "wfc2": wfc2}
        )
    return in_maps


_CACHED_NC = None


def kernel(x, W_ih, W_hh, b_ih, b_hh, W_fc, b_fc):
    global _CACHED_NC
    x = np.asarray(x, np.float32)
    args = [np.asarray(a, np.float32) for a in (W_ih, W_hh, b_ih, b_hh, W_fc, b_fc)]
    if _CACHED_NC is None:
        _CACHED_NC = build_nc()
    nc = _CACHED_NC
    in_maps = make_in_maps(x, *args)
    res = run_bass_kernel_spmd(nc, in_maps, core_ids=list(range(NCORES)))
    b_fc = args[5]
    full = np.empty((1, B, O), np.float32)
    for core in range(NCORES):
        oc = res.results[core]["out"]  # [2*O, NG]
        for g in range(2):
            lo = core * BLOC + g * NG
            full[0, lo : lo + NG, :] = oc[g * O : (g + 1) * O].T + b_fc
    return full


# revision 17
# speedup vs baseline: 1.2639x; 1.2639x over previous
"""Trainium2 Bass kernel for a 1-layer LSTM (B=2048, T=512, I=4, H=64) + FC (O=4).

Sharding: data-parallel over batch across 8 NeuronCores (256 examples/core);
the tiny LSTM/FC weights are replicated.

On-core layout: SBUF partitions carry hidden/gate rows, the free dimension
carries batch.  The 256 local examples form two groups of 128; the groups are
stacked in the partition dimension (group 0 -> rows 0-63, group 1 -> rows
64-127) so ScalarE/VectorE instructions run with all 128 lanes busy.

The recurrent state is a single tile hbuf[128, 128] (both groups' h stacked).
Per step the gate pre-activations are built by PSUM accumulation of two
matmuls per gate chunk:
  mm_x (start=True):  stat Wx2[10, 128]  x  x-slice[10, 128]  (bias+x part)
  mm_h (stop=True):   stat Wh2[128, 128] x  hbuf[128, 128]    (recurrent)
      Wh2 = blockdiag(W_hh_chunk^T, W_hh_chunk^T) so ONE matmul covers both
      groups; only 4 h-matmuls gate the step.
Then on ScalarE: sigmoid(i|f) (strided ACT across the two banks), tanh(g),
sigmoid(o) (in the ScalarE shadow), tanh(c); on VectorE: w = sf*c, u = si*tg,
c = u + w, h = so * tanh(c).

v3 changes vs the 1.247 ms baseline:
  - The whole input x is preloaded into SBUF once (xall, [40, T/4, 128]
    bf16 = 32 KB/partition), eliminating the per-step DMA and its
    sync-sequencer descriptor generation (~750 ns/step of queue work).
  - ScalarE "pad" ops (dummy sigmoids on PSUM scratch) are inserted before
    the two ACT instructions that otherwise start from an idle engine
    (sigmoid(i|f) and tanh(c)).  A gapped ACT start costs ~116 ns extra on
    TRN2 (read-write bubble); a pad sized to end just as the real op's
    dependency lands converts that to a back-to-back start.
"""

from contextlib import ExitStack

import numpy as np

import concourse.bass as bass
import concourse.tile as tile
from concourse import bacc, mybir
from concourse.bass_utils import run_bass_kernel_spmd

F32 = mybir.dt.float32
BF16 = mybir.dt.bfloat16
AF = mybir.ActivationFunctionType

H, I, O = 64, 4, 4
B, T_FULL = 2048, 512
NCORES = 8
BLOC = B // NCORES          # 256 examples per core
NG = 128                    # batch per group (2 groups per core)
KX = 2 * (1 + I)            # 10 rows of x-slice: [1; x_g0; 1; x_g1]
XFOLD = 2                   # timesteps folded into the xall partition dim (bases 0/64)

USE_BF16 = True

# PE issue order of the gate chunks (ids: 0=i, 1=f, 2=g, 3=o)
CHUNKS = (1, 0, 2, 3)

# ACT pad sizes (free-dim elements of the dummy sigmoid); 0 disables.
# pad1 runs between sigmoid(o) and tanh(c); pad2 between tanh(c) and the
# next step's sigmoid(i|f).
PAD1_FD = 16
PAD2_FD = 640


def build_nc(T=T_FULL, use_bf16=None, pad1=None, pad2=None):
    if use_bf16 is None:
        use_bf16 = USE_BF16
    if pad1 is None:
        pad1 = PAD1_FD
    if pad2 is None:
        pad2 = PAD2_FD
    DT = BF16 if use_bf16 else F32
    assert T % XFOLD == 0
    TQ = T // XFOLD
    nc = bacc.Bacc(
        "TRN2",
        target_bir_lowering=False,
        debug=False,
        enable_asserts=False,
        num_devices=NCORES,
    )

    xq = nc.dram_tensor("xq", [128, TQ, NG], DT, kind="ExternalInput")
    wh2 = nc.dram_tensor("wh2", [2 * H, 4, 2 * H], DT, kind="ExternalInput")
    wx2 = nc.dram_tensor("wx2", [128, 4, 2 * H], DT, kind="ExternalInput")
    wfc2 = nc.dram_tensor("wfc2", [2 * H, 2 * O], DT, kind="ExternalInput")
    out = nc.dram_tensor("out", [2 * O, NG], F32, kind="ExternalOutput")

    with tile.TileContext(nc) as tc, ExitStack() as ctx:
        persist = ctx.enter_context(tc.tile_pool(name="persist", bufs=1))
        acts = ctx.enter_context(tc.tile_pool(name="acts", bufs=3))
        temps = ctx.enter_context(tc.tile_pool(name="temps", bufs=3))
        psum = ctx.enter_context(tc.tile_pool(name="psum", bufs=1, space="PSUM"))
        psum1 = ctx.enter_context(tc.tile_pool(name="psum1", bufs=1, space="PSUM"))

        # Whole-input preload plus weights. xall is the big one; it goes on
        # the sync queue, the small weights on the ScalarE HWDGE queue.
        xall = persist.tile([128, TQ, NG], DT, tag="xall")
        nc.sync.dma_start(xall[:], xq[:])
        wh2_sb = persist.tile([2 * H, 4, 2 * H], DT, tag="wh2")
        nc.sync.dma_start(wh2_sb[:], wh2[:])
        # x-weights replicated at partition bases 0/32/64/96 so the matmul's
        # stationary base matches the xall slice base (PE tile-position rule).
        wx2_sb = persist.tile([128, 4, 2 * H], DT, tag="wx2")
        nc.scalar.dma_start(wx2_sb[:], wx2[:])
        wfc2_sb = persist.tile([2 * H, 2 * O], DT, tag="wfc2")
        nc.scalar.dma_start(wfc2_sb[:], wfc2[:])

        # Persistent state: cell state and the double-buffered hidden state.
        c_st = persist.tile([2 * H, NG], DT, tag="c")
        nc.vector.memset(c_st[:], 0.0)
        # Dependency-free source for the ACT pad ops (never written again).
        jnk = persist.tile([2 * H, max(pad1, pad2, 1)], DT, tag="jnk")
        nc.vector.memset(jnk[:], 0.0)
        hbuf = []
        for j in range(2):
            hb = persist.tile([2 * H, NG], DT, tag=f"h{j}")
            nc.vector.memset(hb[:], 0.0)
            hbuf.append(hb)

        def xs(t):
            q = 64 * (t % XFOLD)
            return xall[q : q + KX, t // XFOLD, :]

        for t in range(T):
            hc = hbuf[t % 2]
            hn = hbuf[(t + 1) % 2]

            # PSUM: `start=True` clears the accumulate (has_written) bits of
            # its whole BANK, so each gate chunk gets a private 2 KB bank.
            BK = 512  # fp32 elements per PSUM bank (per partition)
            psIF = psum.tile([2 * H, 2 * BK], F32, tag="psIF")  # i @0, f @512
            psGO = psum.tile([2 * H, 2 * BK], F32, tag="psGO")  # g @0, o @512
            regions = {
                0: psIF[:, 0:NG],            # i
                1: psIF[:, BK : BK + NG],    # f
                2: psGO[:, 0:NG],            # g
                3: psGO[:, BK : BK + NG],    # o
            }

            # x/bias parts: pre-run in the PE's idle window while the
            # h-matmuls wait for h (gated by the prior step's ACT reads of
            # these tiles, which is timing-harmless).
            q = 64 * (t % XFOLD)
            for ch in CHUNKS:
                nc.tensor.matmul(
                    regions[ch],
                    wx2_sb[q : q + KX, ch, :],
                    xs(t),
                    start=True,
                    stop=False,
                )
            # recurrent parts: the 4-matmul wave gating the step.
            for ch in CHUNKS:
                nc.tensor.matmul(
                    regions[ch], wh2_sb[:, ch, :], hc[:], start=False, stop=True
                )

            tg = acts.tile([2 * H, NG], DT, tag="tg")
            sif = acts.tile([2 * H, 2 * NG], DT, tag="sif")
            if pad2 and t > 0:
                padt2 = acts.tile([2 * H, pad2], DT, tag="padt2")
                nc.scalar.activation(padt2[:], jnk[:, 0:pad2], AF.Sigmoid)
            nc.scalar.activation(
                sif[:],
                psIF[:].rearrange("p (b n) -> p b n", b=2)[:, :, 0:NG],
                AF.Sigmoid,
            )
            nc.scalar.activation(tg[:], regions[2], AF.Tanh)
            so = acts.tile([2 * H, NG], DT, tag="so")
            nc.scalar.activation(so[:], regions[3], AF.Sigmoid)

            si = sif[:, 0:NG]
            sf = sif[:, NG : 2 * NG]

            w = temps.tile([2 * H, NG], DT, tag="w")
            nc.vector.tensor_mul(w[:], sf, c_st[:])
            u = temps.tile([2 * H, NG], DT, tag="u")
            nc.vector.tensor_mul(u[:], si, tg[:])
            nc.vector.tensor_add(c_st[:], u[:], w[:])

            if pad1:
                padt1 = acts.tile([2 * H, pad1], DT, tag="padt1")
                nc.scalar.activation(padt1[:], jnk[:, 0:pad1], AF.Sigmoid)
            tcs = acts.tile([2 * H, NG], DT, tag="tc")
            nc.scalar.activation(tcs[:], c_st[:], AF.Tanh)

            nc.vector.tensor_mul(hn[:], so[:], tcs[:])

        # Final FC: one matmul, both groups ([O g0 | O g1] output rows).
        hf = hbuf[T % 2]
        fc_ps = psum1.tile([2 * O, NG], F32, tag="fc")
        nc.tensor.matmul(fc_ps[:], wfc2_sb[:], hf[:], start=True, stop=True)
        fc_sb = temps.tile([2 * O, NG], F32, tag="fcsb")
        nc.vector.tensor_copy(fc_sb[:], fc_ps[:])
        nc.sync.dma_start(out[:], fc_sb[:])

    nc.compile()
    return nc


def prep_weights(W_ih, W_hh, b_ih, b_hh, W_fc, b_fc):
    bsum = (b_ih + b_hh).astype(np.float32)
    wh2 = np.zeros((2 * H, 4, 2 * H), np.float32)
    wx2 = np.zeros((KX, 4, 2 * H), np.float32)
    for ch in range(4):
        r = slice(ch * H, (ch + 1) * H)
        wh2[0:H, ch, 0:H] = W_hh[r].T
        wh2[H:, ch, H:] = W_hh[r].T
        wx2[0, ch, 0:H] = bsum[r]
        wx2[1 : 1 + I, ch, 0:H] = W_ih[r].T
        wx2[1 + I, ch, H:] = bsum[r]
        wx2[2 + I :, ch, H:] = W_ih[r].T
    wfc2 = np.zeros((2 * H, 2 * O), np.float32)
    wfc2[0:H, 0:O] = W_fc.T
    wfc2[H:, O:] = W_fc.T
    wx4 = np.zeros((128, 4, 2 * H), np.float32)
    for qq in range(XFOLD):
        wx4[64 * qq : 64 * qq + KX] = wx2
    return wh2, wx4, wfc2


def make_in_maps(x, W_ih, W_hh, b_ih, b_hh, W_fc, b_fc, T=T_FULL, use_bf16=None):
    import ml_dtypes

    if use_bf16 is None:
        use_bf16 = USE_BF16
    npdt = ml_dtypes.bfloat16 if use_bf16 else np.float32
    wh2, wx2, wfc2 = prep_weights(W_ih, W_hh, b_ih, b_hh, W_fc, b_fc)
    wh2, wx2, wfc2 = (a.astype(npdt) for a in (wh2, wx2, wfc2))
    TQ = T // XFOLD
    in_maps = []
    for core in range(NCORES):
        xc = x[core * BLOC : (core + 1) * BLOC, :T, :]  # [BLOC, T, I]
        xT = np.ascontiguousarray(xc.transpose(1, 2, 0))  # [T, I, BLOC]
        xT2 = np.empty((T, KX, NG), np.float32)
        xT2[:, 0, :] = 1.0
        xT2[:, 1 : 1 + I, :] = xT[:, :, 0:NG]
        xT2[:, 1 + I, :] = 1.0
        xT2[:, 2 + I :, :] = xT[:, :, NG : 2 * NG]
        # Fold 2 consecutive timesteps into the partition dim:
        # xq[64*q + k, u, n] = xT2[u*XFOLD + q, k, n]
        xqa = np.zeros((128, TQ, NG), np.float32)
        folded = xT2.reshape(TQ, XFOLD, KX, NG).transpose(1, 2, 0, 3)
        for qq in range(XFOLD):
            xqa[64 * qq : 64 * qq + KX] = folded[qq]
        in_maps.append(
            {"xq": xqa.astype(npdt), "wh2": wh2, "wx2": wx2, "wfc2": wfc2}
        )
    return in_maps


_CACHED_NC = None


def kernel(x, W_ih, W_hh, b_ih, b_hh, W_fc, b_fc):
    global _CACHED_NC
    x = np.asarray(x, np.float32)
    args = [np.asarray(a, np.float32) for a in (W_ih, W_hh, b_ih, b_hh, W_fc, b_fc)]
    if _CACHED_NC is None:
        _CACHED_NC = build_nc()
    nc = _CACHED_NC
    in_maps = make_in_maps(x, *args)
    res = run_bass_kernel_spmd(nc, in_maps, core_ids=list(range(NCORES)))
    b_fc = args[5]
    full = np.empty((1, B, O), np.float32)
    for core in range(NCORES):
        oc = res.results[core]["out"]  # [2*O, NG]
        for g in range(2):
            lo = core * BLOC + g * NG
            full[0, lo : lo + NG, :] = oc[g * O : (g + 1) * O].T + b_fc
    return full
